# revision 1
# baseline (speedup 1.0000x reference)
"""Trainium2 Bass kernel for nn_BlockAttnRes (block-softmax residual net).

Shapes: embedding [8, 8192, 128] f32, L=16 layers, BLOCK_SIZE=4.
Sharding: batch dim B=8 across 8 cores (1 batch row / core = 8192 tokens).

Per-core: tokens-on-partitions ("row") bf16 state resident in SBUF.
5 row slots: slot0 = emb, slot 1+g = partial of group g (becomes block g+1
at commit). For_i over token tiles (F=512 tokens = 4 blocks of 128),
python-unrolled 16 layers inside, NS=4 tiles interleaved per iteration.

Design (v9, ~2.31 ms vs 2.70 ms v1 baseline):
  - all state bf16 (2x DVE modes, 1-pass PE transposes)
  - partial accumulated in COLUMN layout directly by the W2 matmuls
    (PSUM f32 accumulation across the 4-layer group, no transpose-accum)
  - per-layer partial dot-stat via a PE matmul on the column copy +
    transpose back; sum-of-squares via per-block DVE square+accum rides
  - creation stats (dots for all 16 layers) via one PE matmul per
    stream from the column copy, 32-partition PSUM band per stream
  - weighted sum: per-block fused mult-add stt chains on DVE with
    hsum riding the last op's hardware accumulator
  - softmax normalizer 1/den FOLDED into LayerNorm for l<15:
    LN(u/den) == (u - mu_u) * rsqrt(var_u + eps*den^2) exactly, so no
    reciprocal/normalize pass per layer (only the final layer divides)
  - softmax-exp via tanh identity e^t=(1+T)/(1-T) (gelu ACT table only)
  - rsqrt via int bit-trick seed + Newton iterations (DVE only)
  - LayerNorm affine folded into W1' = diag(g)@W1, b1' = b1 + ln_b@W1
  - W1->gelu h1 PSUM double-buffered (2 banks) so PE/ACT pipeline;
    stat-transpose PSUM shares the xnT/prow bank (disjoint lifetimes)
  - PYTHONHASHSEED pinned for neuronxcc subprocesses: the compiler
    schedule is hash-order sensitive (~±20% swings otherwise)
"""
import contextlib
import ctypes
import os
import sys
import types
from contextlib import ExitStack

os.environ.setdefault("PYTHONHASHSEED", "1")

sys.path.insert(0, "/opt/trn_rl_repo")


def _install_ntff_hook():
    """Provide antenv.axon_hooks (missing in the trimmed repo) so
    run_bass_kernel_spmd(trace=True) can collect NTFF profiles."""
    if "antenv.axon_hooks" in sys.modules:
        return
    try:
        lib = ctypes.CDLL("/opt/axon/libaxon_pjrt.so")
    except OSError:
        return
    if not hasattr(lib, "axon_start_nrt_profile"):
        hook = None
    else:
        lib.axon_start_nrt_profile.argtypes = [
            ctypes.POINTER(ctypes.c_int64), ctypes.c_size_t]
        lib.axon_start_nrt_profile.restype = ctypes.c_int64
        lib.axon_stop_nrt_profile.argtypes = [ctypes.c_char_p]
        lib.axon_stop_nrt_profile.restype = ctypes.c_int64

        @contextlib.contextmanager
        def hook(output_dir, device_ids):
            import jax
            jax.devices()
            if device_ids:
                ids = (ctypes.c_int64 * len(device_ids))(*device_ids)
                rc = lib.axon_start_nrt_profile(ids, len(device_ids))
            else:
                rc = lib.axon_start_nrt_profile(None, 0)
            if rc != 0:
                raise RuntimeError(f"axon_start_nrt_profile rc={rc}")
            try:
                yield
            finally:
                n = lib.axon_stop_nrt_profile(str(output_dir).encode())
                print(f"profile: {n} file(s) -> {output_dir}", file=sys.stderr)

    mod = types.ModuleType("antenv.axon_hooks")
    mod.get_axon_ntff_profile_hook = lambda: hook
    mod.set_axon_ntff_profile_hook = lambda h: None
    sys.modules["antenv.axon_hooks"] = mod

import numpy as np
import ml_dtypes

import concourse.bacc as bacc
import concourse.bass as bass
import concourse.mybir as mybir
from concourse.bass_utils import run_bass_kernel_spmd
from concourse.tile import TileContext
from concourse.masks import make_identity

F32 = mybir.dt.float32
BF16 = mybir.dt.bfloat16
I32 = mybir.dt.int32
ALU = mybir.AluOpType
AF = mybir.ActivationFunctionType
AX = mybir.AxisListType

L = 16
GROUP = 4
D = 128
NBLK = 4                 # 128-token blocks per tile
F = NBLK * 128           # tokens per tile
EPS_RMS = 1e-8
EPS_LN = 1e-5
MAGIC = 0x5F3759DF
N_CORES = 8

_CACHE = {}


def _mkap(base, extra_off, dims):
    """Build an AP from base AP's tensor with partition dim kept and given
    free dims [[stride, count], ...] (element units)."""
    return bass.AP(tensor=base.tensor, offset=base.offset + extra_off,
                   ap=[base.ap[0]] + [list(d) for d in dims])


def _bcast(ap, n):
    """Append a stride-0 inner free dim of size n to an AP."""
    return bass.AP(tensor=ap.tensor, offset=ap.offset,
                   ap=list(ap.ap) + [[0, n]])


def _newton_rsqrt(nc, pool, x, shape, iters=2):
    """y = rsqrt(x) for x [128, *shape] f32 tile (positive). Returns y tile."""
    y = pool.tile([128] + list(shape), F32, tag="nw_y", name="nw_y")
    xi = x.bitcast(I32)
    yi = y.bitcast(I32)
    nc.vector.tensor_scalar(out=yi[:], in0=xi[:], scalar1=1, scalar2=0,
                            op0=ALU.logical_shift_right,
                            op1=ALU.logical_shift_right)
    nc.vector.tensor_scalar(out=yi[:], in0=yi[:], scalar1=-1, scalar2=MAGIC,
                            op0=ALU.mult, op1=ALU.add)
    t = pool.tile([128] + list(shape), F32, tag="nw_t", name="nw_t")
    for _ in range(iters):
        nc.vector.tensor_mul(t[:], y[:], y[:])
        nc.vector.scalar_tensor_tensor(out=t[:], in0=t[:], scalar=-0.5,
                                       in1=x[:], op0=ALU.mult, op1=ALU.mult)
        nc.vector.scalar_tensor_tensor(out=y[:], in0=t[:], scalar=1.5,
                                       in1=y[:], op0=ALU.add, op1=ALU.mult)
    return y


def build(tiles_per_core=16):
    nc = bacc.Bacc("TRN2", target_bir_lowering=False)
    n_tok = tiles_per_core * F

    emb = nc.dram_tensor("emb", [n_tok, D], F32, kind="ExternalInput")
    # wallT1: col l (l<16) = w[l]; col 16 = ones
    wallT1 = nc.dram_tensor("wallT1", [D, L + 1], BF16, kind="ExternalInput")
    w1p = nc.dram_tensor("w1p", [D, L * 2 * 128], BF16, kind="ExternalInput")
    b1p = nc.dram_tensor("b1p", [128, 2 * L], F32, kind="ExternalInput")
    w2p = nc.dram_tensor("w2p", [128, L * 2 * D], BF16, kind="ExternalInput")
    out = nc.dram_tensor("out", [n_tok, D], F32, kind="ExternalOutput")

    emb_v = emb.rearrange("(T b p) d -> T p b d", b=NBLK, p=128)
    out_v = out.rearrange("(T b p) d -> T p b d", b=NBLK, p=128)

    NS = 4 if tiles_per_core % 4 == 0 else (
        2 if tiles_per_core % 2 == 0 else 1)
    NST = 17  # stats rows per stream at creation (16 dots + 1 ms)

    with TileContext(nc) as tc, ExitStack() as es:
        cst = es.enter_context(tc.tile_pool(name="cst", bufs=1))
        identb = cst.tile([128, 128], BF16)
        make_identity(nc, identb[:])
        wallT1_sb = cst.tile([128, L + 1], BF16)
        nc.sync.dma_start(wallT1_sb[:], wallT1[:])
        w1p_sb = cst.tile([128, L, 2, 128], BF16)
        nc.sync.dma_start(w1p_sb[:], w1p[:].rearrange(
            "d (l h m) -> d l h m", l=L, h=2))
        b1p_sb = cst.tile([128, 2 * L], F32)
        nc.sync.dma_start(b1p_sb[:], b1p[:])
        w2p_sb = cst.tile([128, L, 2, D], BF16)
        nc.sync.dma_start(w2p_sb[:], w2p[:].rearrange(
            "m (l k d) -> m l k d", l=L, k=2))

        sp = es.enter_context(tc.tile_pool(name="state", bufs=NS))
        big = es.enter_context(tc.tile_pool(name="big", bufs=NS + 2))
        sml = es.enter_context(tc.tile_pool(name="sml", bufs=12))
        nwp = es.enter_context(tc.tile_pool(name="nw", bufs=12))
        spd = es.enter_context(tc.tile_pool(name="spd", bufs=1))
        pp_par = es.enter_context(tc.tile_pool(name="pp_par", bufs=NS,
                                               space="PSUM"))
        pp_h1 = es.enter_context(tc.tile_pool(name="pp_h1", bufs=1,
                                              space="PSUM"))
        pp_mm = es.enter_context(tc.tile_pool(name="pp_mm", bufs=1,
                                              space="PSUM"))
        pp_xt = es.enter_context(tc.tile_pool(name="pp_xt", bufs=1,
                                              space="PSUM"))

        def stat_matmuls(st, src_col, wcols, mm_ps):
            """Stream k's 32-partition band: mm_ps rows [32k:32k+len] =
            wallT1[:, wcols].T @ src_col (psum outputs must be 32-aligned).
            wcols is an AP over wallT1_sb columns."""
            b0 = 32 * st["k"]
            nrow = wcols.ap[-1][1]
            nc.tensor.matmul(mm_ps[b0:b0 + nrow, :], wcols,
                             src_col[:], start=True, stop=True,
                             tile_position=(0, b0),
                             skip_group_check=True)

        def slot_ssq(sts, slot_idx, tag):
            """Per-token sum of squares of a row slot (square + reduce)."""
            ssq = sml.tile([128, NS, NBLK], F32, tag=tag, name=tag, bufs=3)
            for st in sts:
                k = st["k"]
                sl = st["slots"]
                for blk in range(NBLK):
                    nc.vector.scalar_tensor_tensor(
                        out=st["trash"][:, blk, :],
                        in0=sl[:, slot_idx, blk, :], scalar=1.0,
                        in1=sl[:, slot_idx, blk, :],
                        op0=ALU.bypass, op1=ALU.mult,
                        accum_out=ssq[:, k, blk:blk + 1])
            return ssq

        def transpose_stats(mm_ps, tag):
            """Dots -> row layout [128, NBLK, 128] f32 (lane 32k+j = dot_j
            of stream k)."""
            mm_sb = big.tile([128, F], BF16, tag="mm_sb", name="mm_sb")
            nc.scalar.copy(mm_sb[:], mm_ps[:])
            tt_ps = pp_xt.tile([128, F], BF16, tag="xt", name="tt_ps")
            for c in range(NBLK):
                nc.tensor.matmul(tt_ps[:, c * 128:(c + 1) * 128],
                                 mm_sb[:, c * 128:(c + 1) * 128],
                                 identb[:],
                                 is_transpose=True, start=True, stop=True,
                                 skip_group_check=True)
            row = sml.tile([128, NBLK, 128], F32, tag=tag, name=tag,
                           bufs=3)
            nc.vector.tensor_copy(
                _mkap(row[:], 0, [[1, F]]), tt_ps[:])
            return row

        def creation_finish(sts, s_idx, stats_row, ssq, sh):
            """stats_row [128, NBLK, 128] (lane 32k+j = dot_j, 32k+16 =
            rowsum) + ssq [128, NS, NBLK] -> scaled sdots_all[:, :, s_idx]
            and rs_all[:, :, s_idx]."""
            ns_ = len(sts)
            sa = sh["sdots_all"][:]
            sr = stats_row[:]
            ra = sh["rs_all"][:]
            nc.vector.tensor_copy(
                _mkap(ra, s_idx * NBLK, [[5 * NBLK, ns_], [1, NBLK]]),
                _mkap(sr, 16, [[32, ns_], [128, NBLK]]))
            xs = sml.tile([128, NS, NBLK], F32, tag="xs_cr", name="xs_cr")
            nc.vector.tensor_scalar(
                out=xs[:, 0:ns_],
                in0=ssq[:, 0:ns_],
                scalar1=1.0 / D, scalar2=EPS_RMS,
                op0=ALU.mult, op1=ALU.add)
            rms = _newton_rsqrt(nc, nwp, xs, (NS, NBLK))
            r_ap = rms[:]
            for k in range(ns_):
                # out: sdots_all slice (blk, l) at (k, s_idx)
                nc.vector.scalar_tensor_tensor(
                    out=_mkap(sa, k * 5 * NBLK * L + s_idx * NBLK * L,
                              [[L, NBLK], [1, L]]),
                    in0=_mkap(sr, 32 * k, [[128, NBLK], [1, L]]),
                    scalar=1.0,
                    in1=_mkap(r_ap, k * NBLK, [[1, NBLK], [0, L]]),
                    op0=ALU.bypass, op1=ALU.mult)

        def tile_start(it, k, sh):
            st = {"it": it, "k": k, "sh": sh}
            st["slots"] = sp.tile([128, 5, NBLK, D], BF16, tag="slots",
                                  name="slots")
            st["trash"] = sp.tile([128, NBLK, D], BF16, tag="trash",
                                  name="trash")
            st["partial_ps"] = pp_par.tile([128, F], F32, tag="par",
                                           name="par")
            # emb f32 dram -> bf16 row slot 0 (SWDGE cast)
            nc.gpsimd.dma_start(out=st["slots"][:, 0], in_=emb_v[bass.ds(it, 1)])
            # col copy for creation stats (PE transpose + ACT copy)
            ecolT = pp_xt.tile([128, F], BF16, tag="xt", name="ecolT")
            for blk in range(NBLK):
                nc.tensor.matmul(ecolT[:, blk * 128:(blk + 1) * 128],
                                 st["slots"][:, 0, blk, :], identb[:],
                                 is_transpose=True, start=True, stop=True,
                                 skip_group_check=True)
            ecol = big.tile([128, F], BF16, tag="ecol", name="ecol")
            nc.scalar.copy(ecol[:], ecolT[:])
            st["ecol"] = ecol
            return st

        def emit_layer(sts, l, sh):
            ns_ = len(sts)
            g, j = l // GROUP, l % GROUP
            nsrc = g + 1
            has_p = j > 0
            n = nsrc + (1 if has_p else 0)
            last = l == L - 1
            sdots_all = sh["sdots_all"]

            E_T = sml.tile([128, NS, NBLK, 5], F32, tag="E_T", name="E_T")
            e_ap = E_T[:]
            statics_out = bass.AP(
                tensor=e_ap.tensor, offset=e_ap.offset,
                ap=[e_ap.ap[0], [NBLK * 5, ns_], [1, nsrc], [5, NBLK]])
            nc.scalar.activation(out=statics_out,
                                 in_=sdots_all[:, 0:ns_, 0:nsrc, :, l],
                                 func=AF.Tanh, scale=0.5)

            if has_p:
                pr = sh["pstat_row"][:]
                # pstat_row [128, NBLK, 128]: dot of stream k at lane 32k
                xp = sml.tile([128, NS, NBLK], F32, tag="xp", name="xp")
                nc.vector.tensor_scalar(
                    out=xp[:, 0:ns_],
                    in0=sh["pssq"][:, 0:ns_],
                    scalar1=1.0 / D, scalar2=EPS_RMS,
                    op0=ALU.mult, op1=ALU.add)
                rmsp = _newton_rsqrt(nc, nwp, xp, (NS, NBLK), iters=1)
                lp = sml.tile([128, NS, NBLK], F32, tag="lp", name="lp")
                nc.vector.tensor_mul(
                    lp[:, 0:ns_],
                    _mkap(pr, 0, [[32, ns_], [128, NBLK]]),
                    rmsp[:, 0:ns_])
                nc.scalar.activation(out=E_T[:, 0:ns_, :, nsrc],
                                     in_=lp[:, 0:ns_],
                                     func=AF.Tanh, scale=0.5)

            Ev = E_T[:, 0:ns_, :, 0:n]
            Bt = sml.tile([128, NS, NBLK, 5], F32, tag="B", name="Bt")
            nc.vector.tensor_scalar(out=Bt[:, 0:ns_, :, 0:n], in0=Ev,
                                    scalar1=-1.0, scalar2=-1.0,
                                    op0=ALU.mult, op1=ALU.subtract)
            R = sml.tile([128, NS, NBLK, 5], F32, tag="R", name="R")
            nc.vector.reciprocal(R[:, 0:ns_, :, 0:n], Bt[:, 0:ns_, :, 0:n])
            E = sml.tile([128, NS, NBLK, 5], F32, tag="E", name="E")
            nc.vector.tensor_scalar(out=E[:, 0:ns_, :, 0:n],
                                    in0=R[:, 0:ns_, :, 0:n],
                                    scalar1=2.0, scalar2=-1.0,
                                    op0=ALU.mult, op1=ALU.add)
            den = sml.tile([128, NS, NBLK], F32, tag="den", name="den")
            nc.vector.tensor_reduce(den[:, 0:ns_], E[:, 0:ns_, :, 0:n],
                                    axis=AX.X, op=ALU.add)
            if last:
                # final output must be normalized: wts = E / den
                rd = sml.tile([128, NS, NBLK], F32, tag="rd", name="rd")
                nc.vector.reciprocal(rd[:, 0:ns_], den[:, 0:ns_])
                wts = sml.tile([128, NS, NBLK, 5], F32, tag="wts",
                               name="wts")
                rd_ap = rd[:, 0:ns_]
                nc.vector.scalar_tensor_tensor(
                    out=wts[:, 0:ns_, :, 0:n], in0=E[:, 0:ns_, :, 0:n],
                    scalar=1.0, in1=_bcast(rd_ap, n),
                    op0=ALU.bypass, op1=ALU.mult)
            else:
                # unnormalized u = sum_i E_i V_i; the 1/den normalizer is
                # folded into LayerNorm: LN(u/den) = (u - mu_u) *
                # rsqrt(var_u + eps*den^2), exactly
                wts = E

            def wtsb(k, i):
                # wts for stream k source i, bcast along d: [128, NBLK, D]
                wv = wts[:]
                return bass.AP(tensor=wv.tensor,
                               offset=wv.offset + 20 * k + i,
                               ap=[wv.ap[0], [5, NBLK], [0, D]])

            # weighted sum: per-block fused mult-add chains (DVE),
            # hsum riding the last op's accumulator
            hsum = sml.tile([128, NS, NBLK], F32, tag="hsum", name="hsum")
            hs = []
            for st in sts:
                k = st["k"]
                slots = st["slots"]
                h = big.tile([128, NBLK, D], F32 if last else BF16,
                             tag="h_f32" if last else "h", name="h")
                hs.append(h)
                for blk in range(NBLK):
                    acc = hsum[:, k, blk:blk + 1] if not last else None
                    if n == 1:
                        nc.vector.tensor_scalar(
                            out=h[:, blk, :], in0=slots[:, 0, blk, :],
                            scalar1=wts[:, k, blk, 0:1], scalar2=0.0,
                            op0=ALU.mult, op1=ALU.add, accum_out=acc)
                    else:
                        nc.vector.tensor_scalar(
                            out=h[:, blk, :], in0=slots[:, 0, blk, :],
                            scalar1=wts[:, k, blk, 0:1], scalar2=None,
                            op0=ALU.mult)
                    for i in range(1, n):
                        nc.vector.scalar_tensor_tensor(
                            out=h[:, blk, :], in0=slots[:, i, blk, :],
                            scalar=wts[:, k, blk, i:i + 1],
                            in1=h[:, blk, :],
                            op0=ALU.mult, op1=ALU.add,
                            accum_out=(acc if i == n - 1 else None))
                if last:
                    nc.gpsimd.dma_start(out=out_v[bass.ds(st["it"], 1)],
                                        in_=h[:])
            if last:
                return

            # hssq per block with accumulator rides
            hssq = sml.tile([128, NS, NBLK], F32, tag="hssq", name="hssq")
            for st, h in zip(sts, hs):
                k = st["k"]
                for blk in range(NBLK):
                    nc.vector.scalar_tensor_tensor(
                        out=st["trash"][:, blk, :], in0=h[:, blk, :],
                        scalar=1.0, in1=h[:, blk, :],
                        op0=ALU.bypass, op1=ALU.mult,
                        accum_out=hssq[:, k, blk:blk + 1])

            m2 = sml.tile([128, NS, NBLK], F32, tag="m2", name="m2")
            nc.vector.tensor_mul(m2[:, 0:ns_], hsum[:, 0:ns_], hsum[:, 0:ns_])
            den2e = sml.tile([128, NS, NBLK], F32, tag="den2e",
                             name="den2e")
            nc.vector.scalar_tensor_tensor(
                out=den2e[:, 0:ns_], in0=den[:, 0:ns_], scalar=EPS_LN,
                in1=den[:, 0:ns_], op0=ALU.mult, op1=ALU.mult)
            t1 = sml.tile([128, NS, NBLK], F32, tag="t1", name="t1")
            nc.vector.scalar_tensor_tensor(
                out=t1[:, 0:ns_], in0=hssq[:, 0:ns_], scalar=1.0 / D,
                in1=den2e[:, 0:ns_], op0=ALU.mult, op1=ALU.add)
            xs2 = sml.tile([128, NS, NBLK], F32, tag="xs2", name="xs2")
            nc.vector.scalar_tensor_tensor(
                out=xs2[:, 0:ns_], in0=m2[:, 0:ns_], scalar=-1.0 / (D * D),
                in1=t1[:, 0:ns_], op0=ALU.mult, op1=ALU.add)
            s_ln = _newton_rsqrt(nc, nwp, xs2, (NS, NBLK), iters=1)
            mu = sml.tile([128, NS, NBLK], F32, tag="mu", name="mu")
            nc.vector.tensor_scalar_mul(mu[:, 0:ns_], hsum[:, 0:ns_], 1.0 / D)

            # xn = (h - mu) * s per block (fused dual-scalar ts)
            for st, h in zip(sts, hs):
                k = st["k"]
                xn = big.tile([128, NBLK, D], BF16, tag="xn", name="xn")
                for blk in range(NBLK):
                    nc.vector.tensor_scalar(
                        out=xn[:, blk, :], in0=h[:, blk, :],
                        scalar1=mu[:, k, blk:blk + 1],
                        scalar2=s_ln[:, k, blk:blk + 1],
                        op0=ALU.subtract, op1=ALU.mult)
                xnT_ps = pp_xt.tile([128, F], BF16, tag="xt", name="xnT_ps")
                for blk in range(NBLK):
                    nc.tensor.matmul(xnT_ps[:, blk * 128:(blk + 1) * 128],
                                     xn[:, blk, :], identb[:],
                                     is_transpose=True, start=True, stop=True,
                                     skip_group_check=True)
                xn_col = big.tile([128, F], BF16, tag="xn_col", name="xn_col")
                nc.scalar.copy(xn_col[:], xnT_ps[:])
                st["xn_col"] = xn_col

            # MLP: W1 -> gelu -> W2 accumulating into column partial PSUM
            # (h1 double-buffered so consecutive W1 matmuls overlap gelus)
            for st in sts:
                G = []
                for half in range(2):
                    h1 = pp_h1.tile([128, F], F32, tag="h1", name="h1",
                                    bufs=2)
                    nc.tensor.matmul(h1[:], w1p_sb[:, l, half, :],
                                     st["xn_col"][:], start=True, stop=True,
                                     skip_group_check=True)
                    gh = big.tile([128, F], BF16, tag=f"g{half}", name="gh")
                    nc.scalar.activation(
                        gh[:], h1[:], AF.Gelu,
                        bias=b1p_sb[:, 2 * l + half:2 * l + half + 1])
                    G.append(gh)
                for kh in range(2):
                    nc.tensor.matmul(
                        st["partial_ps"][:], w2p_sb[:, l, kh, :], G[kh][:],
                        start=(j == 0 and kh == 0),
                        stop=((j == GROUP - 1 or l == L - 2) and kh == 1),
                        skip_group_check=True)
                pcol = big.tile([128, F], BF16, tag="pcol", name="pcol")
                nc.vector.tensor_copy(pcol[:], st["partial_ps"][:])
                st["pcol"] = pcol
                # partial row slot via PE transpose + DVE copy
                prow_ps = pp_xt.tile([128, F], BF16, tag="xt",
                                     name="prow_ps")
                for blk in range(NBLK):
                    nc.tensor.matmul(prow_ps[:, blk * 128:(blk + 1) * 128],
                                     pcol[:, blk * 128:(blk + 1) * 128],
                                     identb[:],
                                     is_transpose=True, start=True, stop=True,
                                     skip_group_check=True)
                nc.vector.tensor_copy(
                    _mkap(st["slots"][:], (g + 1) * NBLK * D, [[1, F]]),
                    prow_ps[:])

            # stats for next layer (partial) or creation (commit)
            mm_ps = pp_mm.tile([128, F], F32, tag="mm", name="mm_ps")
            if j < GROUP - 1:
                wv = wallT1_sb[:]
                wcols = bass.AP(tensor=wv.tensor, offset=wv.offset + l + 1,
                                ap=[wv.ap[0], [L - (l + 1), 2]])
                for st in sts:
                    stat_matmuls(st, st["pcol"], wcols, mm_ps)
                sh["pstat_row"] = transpose_stats(mm_ps, "pstat_row")
                sh["pssq"] = slot_ssq(sts, g + 1, "pssq")
            else:
                for st in sts:
                    stat_matmuls(st, st["pcol"], wallT1_sb[:, 0:L + 1], mm_ps)
                stats_row = transpose_stats(mm_ps, "stats_row")
                ssq = slot_ssq(sts, g + 1, "cssq")
                creation_finish(sts, g + 1, stats_row, ssq, sh)

        spd_pool = spd
        with tc.For_i(0, tiles_per_core // NS, 1,
              hint_engines=(mybir.EngineType.DVE,
                            mybir.EngineType.Activation,
                            mybir.EngineType.PE,
                            mybir.EngineType.Pool)) as it0:
            sh = {}
            sh["sdots_all"] = spd_pool.tile([128, NS, 5, NBLK, L], F32,
                                            tag="sdots_all", name="sdots_all")
            sh["rs_all"] = spd_pool.tile([128, NS, 5, NBLK], F32,
                                         tag="rs_all", name="rs_all")
            sts = [tile_start(it0 * NS + k, k, sh) for k in range(NS)]
            # emb creation stats (memset clears stale psum in unused rows so
            # the transpose/selection matmuls never touch NaN garbage)
            mm_ps = pp_mm.tile([128, F], F32, tag="mm", name="mm_ps")
            nc.vector.memset(mm_ps[:], 0.0)
            for st in sts:
                stat_matmuls(st, st["ecol"], wallT1_sb[:, 0:L + 1], mm_ps)
            stats_row = transpose_stats(mm_ps, "stats_row")
            ssq = slot_ssq(sts, 0, "cssq")
            creation_finish(sts, 0, stats_row, ssq, sh)
            for l in range(L):
                emit_layer(sts, l, sh)

    nc.finalize()
    return nc


def _prep_consts(w, ln_g, ln_b, W1, b1, W2):
    bf = ml_dtypes.bfloat16
    W1p = ln_g[:, :, None] * W1                                   # diag(g) @ W1
    b1p = b1 + np.einsum("ld,ldm->lm", ln_b, W1)                  # b1 + ln_b @ W1
    w1p = np.ascontiguousarray(W1p.transpose(1, 0, 2)).reshape(D, L * 2 * 128)
    b1p_sb = b1p.reshape(L, 2, 128).transpose(2, 0, 1).reshape(128, 2 * L)
    w2p = W2.reshape(L, 2, 128, D).transpose(2, 0, 1, 3)
    w2p = np.ascontiguousarray(w2p).reshape(128, L * 2 * D)
    wallT1 = np.concatenate([w.T, np.ones((D, 1), np.float32)], axis=1)
    return {
        "wallT1": np.ascontiguousarray(wallT1).astype(bf),
        "w1p": w1p.astype(bf),
        "b1p": np.ascontiguousarray(b1p_sb).astype(np.float32),
        "w2p": w2p.astype(bf),
    }


def kernel(embedding, w, ln_g, ln_b, W1, b1, W2, b2, _tiles=16, _trace=False):
    if _trace:
        _install_ntff_hook()
    B, T, Dd = embedding.shape
    assert Dd == D
    n_tok = _tiles * F

    key = ("k", _tiles)
    if key not in _CACHE:
        _CACHE[key] = build(_tiles)
    nc = _CACHE[key]

    assert np.all(np.asarray(b2) == 0.0), "nonzero b2 unsupported"
    consts = _prep_consts(np.asarray(w, np.float32),
                          np.asarray(ln_g, np.float32),
                          np.asarray(ln_b, np.float32),
                          np.asarray(W1, np.float32),
                          np.asarray(b1, np.float32),
                          np.asarray(W2, np.float32))
    emb_full = np.asarray(embedding, np.float32).reshape(B * T, D)

    per_core = B * T // N_CORES
    in_maps = []
    for c in range(N_CORES):
        shard = emb_full[c * per_core:(c + 1) * per_core][:n_tok]
        in_maps.append({"emb": np.ascontiguousarray(shard), **consts})

    res = run_bass_kernel_spmd(nc, in_maps, core_ids=list(range(N_CORES)),
                               trace=_trace)
    outs = [res.results[c]["out"] for c in range(N_CORES)]
    full = np.stack(outs).reshape(N_CORES, n_tok, D)
    kernel.last_exec_ns = getattr(res, "exec_time_ns", None)
    kernel.last_mean_ns = getattr(res, "mean_exec_time_ns", None)
    if n_tok == per_core:
        return full.reshape(B, T, D)
    return full  # debug partial run

